# revision 8
# baseline (speedup 1.0000x reference)
"""Trainium2 Bass kernel for AttentionPooling (ragged span attention pooling).

Math restructuring (vs the reference's gather-then-project):
  - K/V projections are computed once per unique token, not per gathered span
    token.  The query is a single shared vector, so per-span softmax
    factorizes:
        attn[s,n,l] = e[start_s+l, n] / Z[s,n],   e[t,n] = exp(q_n . k_{t,n} / 8)
        Z[s,n]      = sum_{t in span_s} e[t,n]
    Hence  ctx[s] = (1/Z[s]) * sum_t W[t,s] * (e[t] (x) V[t])  with the SAME 0/1
    banded window matrix W for all heads -> one dense matmul per core.
  - Spans are sorted by start and split contiguously, so each core only needs
    a TW=256-token window of x -> V'/ctx matmuls and the xT DMA halve.
  - Everything runs feature-major (features on partitions, spans on free dim).
  - FFN2 + LN2 feature-major: residual is a DVE add (no PE transposes), LN2
    variance via ones-column matmul; W2 columns are centered host-side so the
    LN2 mean is exactly zero.  Output leaves feature-major bf16, transposed on
    host.
  - Masked-out spans are compacted away host-side (SPC=416 spans/core).
  - DVE/Pool engines split the elementwise work; dummy PE matmuls hold the
    clock at phase boundaries (p-state ramp).

Sharding: sorted unmasked spans of batch b -> cores 4b..4b+3; weights
replicated.
"""

import sys
import numpy as np

if "/opt/trn_rl_repo" not in sys.path:
    sys.path.insert(0, "/opt/trn_rl_repo")

B, T, S, H, L, NH = 2, 512, 2048, 768, 32, 12
HD = H // NH            # 64
INTERMED = 4 * H        # 3072
NCORES = 8
HC = H // 128            # 6 feature chunks
IC = INTERMED // 128     # 24 intermediate chunks
VW = H + NH              # 780: [e-scaled V | e]
EPS = 1e-5

_COMPILED = {}           # (spc, tw) -> compiled Bass


def _build(spc, tw, gb_identity, ib_zero, b2_zero):
    import concourse.bacc as bacc
    import concourse.tile as tile
    from concourse import mybir
    from concourse.alu_op_type import AluOpType as Op

    f32 = mybir.dt.float32
    f32r = mybir.dt.float32r
    bf16 = mybir.dt.bfloat16
    Act = mybir.ActivationFunctionType

    SPC = spc
    TW = tw
    TCW = TW // 128

    nc = bacc.Bacc("TRN2", target_bir_lowering=False, debug=False, num_devices=NCORES)

    def din(name, shape, dt=f32):
        return nc.dram_tensor(name, list(shape), dt, kind="ExternalInput").ap()

    # --- big packed inputs (few DMAs, partition-major) ---
    xT = din("xT", [128, HC, TW], bf16)       # x.T window: [p, c, t_local]
    wvl = din("wvl", [128, HC, VW], bf16)     # [p, c, n] = [Wv.T | wq2.T]
    wout = din("wout", [128, HC, H], bf16)    # centered Wout.T: [p, c, m]
    w1p_ = din("w1p", [128, IC, HC, 128], bf16)   # w1p[p,i,c,m] = W1[i*128+m, c*128+p]
    w2p_ = din("w2p", [128, IC, HC, 128], bf16)   # w2p[p,i,c,m] = W2c[c*128+m, i*128+p]
    # --- small consts ---
    sem = din("sem", [1, 3 * SPC + 128])      # [starts|ends|mask|ones] (local)
    onesc = din("onesc", [128, 1])            # ones column (f32r lhsT)
    cpk = din("cpk", [128, TCW + HC + HC + IC], f32)  # [iota|b1c|b2c|b1]
    ssel = din("ssel", [NH, H])               # head selector
    if not ib_zero:
        bvl = din("bvl", [1, VW], bf16)
    if not gb_identity:
        gcol = din("gcol", [128, HC])
        bcol = din("bcol", [128, HC])

    out = nc.dram_tensor("out", [128, HC, SPC], bf16, kind="ExternalOutput").ap()

    with tile.TileContext(nc) as tc:
        with (
            tc.tile_pool(name="consts", bufs=1) as cp,
            tc.tile_pool(name="big", bufs=1) as bp,
            tc.tile_pool(name="x1keep", bufs=1) as x1p,
            tc.tile_pool(name="psKW", bufs=1, space="PSUM") as psKW,
        ):
            # ---- DMA plan (3 queues; transfers are async per queue) ----
            sem_sb = cp.tile([1, 3 * SPC + 128], f32r)
            nc.scalar.dma_start(sem_sb[:], sem.bitcast(f32r))
            onescol = cp.tile([128, 1], f32r)
            nc.scalar.dma_start(onescol[:], onesc.bitcast(f32r))
            cpk_sb = cp.tile([128, TCW + HC + HC + IC], f32)
            nc.scalar.dma_start(cpk_sb[:], cpk)
            ssel_sb = cp.tile([NH, H], f32r)
            nc.scalar.dma_start(ssel_sb[:], ssel.bitcast(f32r))
            if not ib_zero:
                bvl_sb = cp.tile([1, VW], bf16)
                nc.scalar.dma_start(bvl_sb[:], bvl)
            if not gb_identity:
                gcol_sb = cp.tile([128, HC], f32)
                nc.scalar.dma_start(gcol_sb[:], gcol)
                bcol_sb = cp.tile([128, HC], f32)
                nc.scalar.dma_start(bcol_sb[:], bcol)
            iota_sb = cpk_sb[:, 0:TCW]
            b1ccol = cpk_sb[:, TCW:TCW + HC]
            b2ccol = cpk_sb[:, TCW + HC:TCW + 2 * HC]
            b1col = cpk_sb[:, TCW + 2 * HC:TCW + 2 * HC + IC]
            starts_r = sem_sb[:, 0:SPC]
            ends_r = sem_sb[:, SPC:2 * SPC]
            mask_r = sem_sb[:, 2 * SPC:3 * SPC]
            onesr = sem_sb[:, 3 * SPC:3 * SPC + 128]

            # gpsimd queue: xT (V'-critical), W1 i=0..15, W2 c=2,3
            xT_sb = bp.tile([128, HC, TW], bf16)
            nc.gpsimd.dma_start(xT_sb[:], xT)
            w1_sb = bp.tile([128, IC, HC, 128], bf16)
            w2_sb = bp.tile([128, IC, HC, 128], bf16)
            nc.gpsimd.dma_start(w1_sb[:, 0:8], w1p_[:, 0:8])
            nc.gpsimd.dma_start(w1_sb[:, 8:16], w1p_[:, 8:16])
            nc.gpsimd.dma_start(w2_sb[:, :, 2:4], w2p_[:, :, 2:4])
            # scalar queue: wvl (V'-critical) after the tiny consts
            wvl_sb = bp.tile([128, HC, VW], bf16)
            nc.scalar.dma_start(wvl_sb[:], wvl)
            # sync queue: wout, w1 tail, w2 c=0,1 then c=4,5
            wout_sb = bp.tile([128, HC, H], bf16)
            nc.sync.dma_start(wout_sb[:], wout)
            nc.sync.dma_start(w1_sb[:, 16:24], w1p_[:, 16:24])
            nc.sync.dma_start(w2_sb[:, :, 0:2], w2p_[:, :, 0:2])
            nc.sync.dma_start(w2_sb[:, :, 4:6], w2p_[:, :, 4:6])

            # engine-side constants (no DMA)
            eps1 = cp.tile([1, 1], f32)
            nc.vector.memset(eps1, EPS)
            wf = cp.tile([1, 128], f32)
            nc.vector.memset(wf, 1.0)

            kw = psKW.tile([128, 128], f32, tag="kw")

            def keep_warm(n):
                for k in range(n):
                    nc.tensor.matmul(kw[:], wf[:], wf[:],
                                     start=(k == 0), stop=(k == n - 1))

            x1T = x1p.tile([128, HC, SPC], f32)    # LN1 output, feature-major
            x1Tb = x1p.tile([128, HC, SPC], bf16)  # bf16 copy for FFN1 rhs
            h1s = x1p.tile([128, IC, SPC], bf16)   # relu(FFN1), feature-major

            # ---------------- attention (feature-major) ----------------
            abp_cm = tc.tile_pool(name="attn_big", bufs=1)
            abp = abp_cm.__enter__()
            with (
                tc.tile_pool(name="attn_s", bufs=2) as asml,
                tc.tile_pool(name="psA", bufs=2, space="PSUM") as psA,
                tc.tile_pool(name="psSE", bufs=1, space="PSUM") as psSE,
                tc.tile_pool(name="psSE2", bufs=1, space="PSUM") as psSE2,
            ):
                # warm up the PE (p-state ramp) while the first loads land
                keep_warm(14)
                # broadcast span starts/ends across partitions on the PE
                startsB = psSE.tile([128, SPC], f32, tag="sb", name="startsB")
                nc.tensor.matmul(startsB[:], onesr, starts_r,
                                 start=True, stop=True)
                endsB = psSE2.tile([128, SPC], f32, tag="eb", name="endsB")
                nc.tensor.matmul(endsB[:], onesr, ends_r,
                                 start=True, stop=True)

                ve = abp.tile([128, TCW, VW], bf16)   # [e*V | e], token-major
                wt = abp.tile([128, TCW, SPC], bf16)  # W[t, s] 0/1 window
                ctxN = abp.tile([128, HC, SPC], bf16)  # normalized ctx

                # W[t, s] = (start_s <= t) & (t < end_s); on DVE before V'
                for t in range(TCW):
                    lt = asml.tile([128, SPC], f32, tag="uexp", name=f"lt{t}")
                    nc.vector.tensor_scalar(
                        lt[:], endsB[:], iota_sb[:, t:t + 1], None, Op.is_gt)
                    nc.vector.scalar_tensor_tensor(
                        wt[:, t, :], startsB[:], iota_sb[:, t:t + 1], lt[:],
                        Op.is_le, Op.mult)

                # V' = x @ [Wv.T | wq2.T]  (out: token-major, 780 wide)
                for t in range(TCW):
                    vp = psA.tile([128, VW], f32, tag="vp")
                    for lo, hi in ((0, 512), (512, VW)):
                        for c in range(HC):
                            nc.tensor.matmul(
                                vp[:, lo:hi],
                                xT_sb[:, c, t * 128:(t + 1) * 128],
                                wvl_sb[:, c, lo:hi],
                                start=(c == 0),
                                stop=(c == HC - 1) if ib_zero else False,
                            )
                        if not ib_zero:
                            onesb = asml.tile([1, 128], bf16, tag="onesb")
                            nc.vector.memset(onesb, 1.0)
                            nc.tensor.matmul(
                                vp[:, lo:hi], onesb[:], bvl_sb[:, lo:hi],
                                start=False, stop=True,
                            )
                    # e = exp(logits) into ve[:, t, 768:780]
                    nc.scalar.activation(ve[:, t, H:VW], vp[:, H:VW], Act.Exp)
                    # ve[:, t, :768] = V * e (vector/pool alternate per t)
                    e_b = ve[:, t, H:VW].unsqueeze(2).broadcast_to([128, NH, HD])
                    nc.vector.tensor_tensor(
                        ve[:, t, 0:H].rearrange("p (n d) -> p n d", d=HD),
                        vp[:, 0:H].rearrange("p (n d) -> p n d", d=HD),
                        e_b, Op.mult,
                    )

            # ---- Z / u / ctx / out_proj / LN1 (fresh PSUM layout) ----
            with (
                tc.tile_pool(name="attn2_s", bufs=2) as asml,
                tc.tile_pool(name="psZ", bufs=1, space="PSUM") as psZ,
                tc.tile_pool(name="psU", bufs=2, space="PSUM") as psU,
                tc.tile_pool(name="psB", bufs=2, space="PSUM") as psB,
                tc.tile_pool(name="psS", bufs=1, space="PSUM") as psS,
            ):
                # Z[n, s] then u = 1/max(Z, tiny)
                zp = psZ.tile([NH, SPC], f32, tag="zp")
                for t in range(TCW):
                    nc.tensor.matmul(
                        zp[:], ve[:, t, H:VW], wt[:, t, :],
                        start=(t == 0), stop=(t == TCW - 1))
                keep_warm(8)
                zc = asml.tile([NH, SPC], f32, tag="zc")
                nc.vector.tensor_scalar(zc[:], zp[:], 1e-6, None, Op.max)
                uf = asml.tile([NH, SPC], f32, tag="uexp", name="uf")
                nc.vector.reciprocal(uf[:], zc[:])
                u_sb = asml.tile([NH, SPC], f32r, tag="u")
                with nc.allow_low_precision(reason="f32r is fp32-width"):
                    nc.vector.tensor_copy(u_sb[:], uf[:])

                # ctxN[h', s] = (sum_t ve[t, h'] * W[t, s]) * u[head(h'), s]
                for c in range(HC):
                    up = psU.tile([128, SPC], f32, tag="up")
                    nc.tensor.matmul(
                        up[:], ssel_sb[:, c * 128:(c + 1) * 128], u_sb[:],
                        start=True, stop=True)
                    uexp = asml.tile([128, SPC], f32, tag="uexp")
                    nc.scalar.activation(uexp[:], up[:], Act.Identity)
                    cp_ = psB.tile([128, SPC], f32, tag="mmS")
                    for t in range(TCW):
                        nc.tensor.matmul(
                            cp_[:], ve[:, t, c * 128:(c + 1) * 128],
                            wt[:, t, :],
                            start=(t == 0), stop=(t == TCW - 1))
                    nc.vector.tensor_tensor(ctxN[:, c, :], cp_[:], uexp[:],
                                            Op.mult)

                # out_proj (centered weights) + LN1, feature-major
                ycs = abp.tile([128, HC, SPC], f32)
                varp = psS.tile([1, SPC], f32, tag="small")
                for m in range(HC):
                    aop = psB.tile([128, SPC], f32, tag="mmS")
                    for c in range(HC):
                        nc.tensor.matmul(
                            aop[:], wout_sb[:, c, m * 128:(m + 1) * 128],
                            ctxN[:, c, :],
                            start=(c == 0), stop=(c == HC - 1))
                    # yc = aop + centered (out_b + query) bias on ACT
                    nc.scalar.activation(ycs[:, m, :], aop[:], Act.Identity,
                                         bias=b1ccol[:, m:m + 1])
                for m in range(HC):
                    sq = asml.tile([128, SPC], f32r, tag="sq")
                    with nc.allow_low_precision(reason="f32r is fp32-width"):
                        nc.vector.tensor_tensor(sq[:], ycs[:, m, :],
                                                ycs[:, m, :], Op.mult)
                    nc.tensor.matmul(
                        varp[:], onescol[:], sq[:],
                        start=(m == 0), stop=(m == HC - 1))
                keep_warm(16)
                sd = asml.tile([1, SPC], f32, tag="sd")
                nc.scalar.activation(sd[:], varp[:], Act.Sqrt,
                                     bias=eps1[:], scale=1.0 / H)
                rf = asml.tile([1, SPC], f32, tag="rf")
                nc.vector.reciprocal(rf[:], sd[:])
                rstd = asml.tile([1, SPC], f32r, tag="rstd")
                with nc.allow_low_precision(reason="f32r is fp32-width"):
                    nc.vector.tensor_copy(rstd[:], rf[:])
                rp = psU.tile([128, SPC], f32, tag="up", name="rp")
                nc.tensor.matmul(rp[:], onesr, rstd[:],
                                 start=True, stop=True)
                rpS = asml.tile([128, SPC], f32, tag="rpS")
                nc.scalar.activation(rpS[:], rp[:], Act.Identity)
                keep_warm(8)
                # x1Tb (bf16, FFN1-critical) on DVE; x1T (f32) on Pool
                for m in range(HC):
                    if gb_identity:
                        with nc.allow_low_precision(reason="bf16 store"):
                            nc.vector.tensor_tensor(x1Tb[:, m, :], ycs[:, m, :],
                                                    rp[:], Op.mult)
                        nc.gpsimd.tensor_tensor(x1T[:, m, :], ycs[:, m, :],
                                                rpS[:], Op.mult)
                    else:
                        tmp = asml.tile([128, SPC], f32, tag="uexp", name="tmp")
                        nc.vector.tensor_tensor(tmp[:], ycs[:, m, :], rp[:],
                                                Op.mult)
                        nc.scalar.activation(x1T[:, m, :], tmp[:], Act.Identity,
                                             scale=gcol_sb[:, m:m + 1],
                                             bias=bcol_sb[:, m:m + 1])
                        with nc.allow_low_precision(reason="bf16 store"):
                            nc.vector.tensor_copy(x1Tb[:, m, :], x1T[:, m, :])

            abp_cm.__exit__(None, None, None)

            # ---------------- FFN1 (feature-major) ----------------
            with tc.tile_pool(name="psH", bufs=3, space="PSUM") as psH:
                for i in range(IC):
                    h1p = psH.tile([128, SPC], f32, tag="h1p")
                    for c in range(HC):
                        nc.tensor.matmul(
                            h1p[:], w1_sb[:, i, c, :], x1Tb[:, c, :],
                            start=(c == 0), stop=(c == HC - 1))
                    nc.scalar.activation(h1s[:, i, :], h1p[:], Act.Relu,
                                         bias=b1col[:, i:i + 1])

            # ---------------- FFN2 + LN2, feature-major ----------------
            with (
                tc.tile_pool(name="ffn2_s", bufs=2) as fsml,
                tc.tile_pool(name="outp", bufs=1) as op_,
                tc.tile_pool(name="psF", bufs=2, space="PSUM") as psF,
                tc.tile_pool(name="psV", bufs=1, space="PSUM") as psV,
            ):
                zt = op_.tile([128, HC, SPC], f32)    # z = h2c + x1 (centered)
                obf = op_.tile([128, HC, SPC], bf16)  # final output
                var2 = psV.tile([1, SPC], f32, tag="v2")
                for c in range(HC):
                    h2p = psF.tile([128, SPC], f32, tag="h2")
                    for i in range(IC):
                        nc.tensor.matmul(
                            h2p[:], w2_sb[:, i, c, :], h1s[:, i, :],
                            start=(i == 0), stop=(i == IC - 1))
                    # residual add (and centered b2 if present)
                    if b2_zero:
                        nc.vector.tensor_tensor(zt[:, c, :], h2p[:],
                                                x1T[:, c, :], Op.add)
                    else:
                        zh = fsml.tile([128, SPC], f32, tag="zh")
                        nc.scalar.activation(zh[:], h2p[:], Act.Identity,
                                             bias=b2ccol[:, c:c + 1])
                        nc.vector.tensor_tensor(zt[:, c, :], zh[:],
                                                x1T[:, c, :], Op.add)
                    sq2 = fsml.tile([128, SPC], f32r, tag="sq2")
                    with nc.allow_low_precision(reason="f32r is fp32-width"):
                        nc.gpsimd.tensor_tensor(sq2[:], zt[:, c, :],
                                                zt[:, c, :], Op.mult)
                    nc.tensor.matmul(
                        var2[:], onescol[:], sq2[:],
                        start=(c == 0), stop=(c == HC - 1))
                keep_warm(16)
                # rstd2 row with mask folded; broadcast via PE
                sd2 = fsml.tile([1, SPC], f32, tag="sd2")
                nc.scalar.activation(sd2[:], var2[:], Act.Sqrt,
                                     bias=eps1[:], scale=1.0 / H)
                rf2 = fsml.tile([1, SPC], f32, tag="rf2")
                nc.vector.reciprocal(rf2[:], sd2[:])
                rm2 = fsml.tile([1, SPC], f32r, tag="rm2")
                with nc.allow_low_precision(reason="f32r is fp32-width"):
                    nc.vector.tensor_tensor(rm2[:], rf2[:],
                                            mask_r.bitcast(f32), Op.mult)
                rp2 = psF.tile([128, SPC], f32, tag="h2", name="rp2")
                nc.tensor.matmul(rp2[:], onesr, rm2[:],
                                 start=True, stop=True)
                rp2S = fsml.tile([128, SPC], f32, tag="rp2S")
                nc.scalar.activation(rp2S[:], rp2[:], Act.Identity)
                for c in range(HC):
                    eng = nc.vector if c % 2 == 0 else nc.gpsimd
                    src_rp = rp2 if c % 2 == 0 else rp2S
                    with nc.allow_low_precision(reason="bf16 output store"):
                        eng.tensor_tensor(obf[:, c, :], zt[:, c, :],
                                          src_rp[:], Op.mult)
                    if c == 1:
                        nc.sync.dma_start(out[:, 0:2, :], obf[:, 0:2, :])
                    elif c == 3:
                        nc.gpsimd.dma_start(out[:, 2:4, :], obf[:, 2:4, :])
                nc.sync.dma_start(out[:, 4:6, :], obf[:, 4:6, :])
    nc.compile()
    return nc


def _plan_cores(span_ids, masks):
    """Sort unmasked spans by start; contiguous quarters per core."""
    starts_all = span_ids[..., 0].astype(np.int64)
    plan = []
    for b in range(B):
        idxs = np.where(masks[b])[0]
        order = np.argsort(starts_all[b, idxs], kind="stable")
        idxs = idxs[order]
        for pj in np.array_split(idxs, NCORES // B):
            plan.append((b, pj))
    return plan


def _host_prepare(inputs, spc, tw):
    """Host-side packing: compaction, weight centering, layout packing."""
    import ml_dtypes
    bf = ml_dtypes.bfloat16

    tr = np.asarray(inputs["token_reps"], dtype=np.float32)
    span_ids = np.asarray(inputs["span_ids"]).astype(np.int64)
    masks = np.asarray(inputs["span_masks"]).astype(bool)
    pe = np.asarray(inputs["pe"], dtype=np.float32)
    q0 = np.asarray(inputs["dummy_query"], dtype=np.float32)
    in_w = np.asarray(inputs["in_proj_w"], dtype=np.float32)
    in_b = np.asarray(inputs["in_proj_b"], dtype=np.float32)
    wo = np.asarray(inputs["out_proj_w"], dtype=np.float32)
    bo = np.asarray(inputs["out_proj_b"], dtype=np.float32)
    g = np.asarray(inputs["norm_g"], dtype=np.float32)
    bb = np.asarray(inputs["norm_b"], dtype=np.float32)
    w1 = np.asarray(inputs["ffn_w1"], dtype=np.float32)
    b1 = np.asarray(inputs["ffn_b1"], dtype=np.float32)
    w2 = np.asarray(inputs["ffn_w2"], dtype=np.float32)
    b2 = np.asarray(inputs["ffn_b2"], dtype=np.float32)

    TCW = tw // 128
    Wq, Wk, Wv = in_w[0:H], in_w[H:2 * H], in_w[2 * H:3 * H]
    bq, bk, bv = in_b[0:H], in_b[H:2 * H], in_b[2 * H:3 * H]

    q = q0 @ Wq.T + bq
    qs = (q / np.sqrt(HD)).astype(np.float32)
    wq2 = np.stack([qs[n * HD:(n + 1) * HD] @ Wk[n * HD:(n + 1) * HD]
                    for n in range(NH)])                      # (12, 768)
    constv = np.array([qs[n * HD:(n + 1) * HD] @ bk[n * HD:(n + 1) * HD]
                       for n in range(NH)], dtype=np.float32)

    wvl = np.concatenate([Wv.T, wq2.T], axis=1).astype(np.float32)   # (768, 780)
    bvl = np.concatenate([bv, constv])[None, :].astype(np.float32)   # (1, 780)

    wout_c = wo - wo.mean(axis=0, keepdims=True)
    b1c_full = bo + q0
    b1c = (b1c_full - b1c_full.mean()).astype(np.float32)
    w2c = w2 - w2.mean(axis=0, keepdims=True)                       # (768, 3072)
    b2c = (b2 - b2.mean()).astype(np.float32)

    ssel = np.zeros((NH, H), dtype=np.float32)
    for n in range(NH):
        ssel[n, n * HD:(n + 1) * HD] = 1.0
    iota = (np.arange(128, dtype=np.float32)[:, None]
            + 128.0 * np.arange(TCW, dtype=np.float32)[None, :])

    x = tr + pe[None, :T]                                          # (B, T, H)
    xT_full = [np.ascontiguousarray(
        x[b].T.reshape(HC, 128, T).transpose(1, 0, 2)).astype(bf)
        for b in range(B)]                                         # [128,HC,T]
    wvl_pk = np.ascontiguousarray(
        wvl.reshape(HC, 128, VW).transpose(1, 0, 2)).astype(bf)
    wout_pk = np.ascontiguousarray(
        wout_c.T.reshape(HC, 128, H).transpose(1, 0, 2)).astype(bf)
    w1pk = np.ascontiguousarray(
        w1.reshape(IC, 128, HC, 128).transpose(3, 0, 2, 1)).astype(bf)
    w2pk = np.ascontiguousarray(
        w2c.reshape(HC, 128, IC, 128).transpose(3, 2, 0, 1)).astype(bf)

    cpk = np.concatenate([
        iota,
        b1c.reshape(HC, 128).T,
        b2c.reshape(HC, 128).T,
        b1.reshape(IC, 128).T,
    ], axis=1).astype(np.float32)

    shared = dict(wvl=wvl_pk, wout=wout_pk, w1p=w1pk, w2p=w2pk,
                  cpk=cpk, ssel=ssel,
                  onesc=np.ones((128, 1), dtype=np.float32))
    if not np.allclose(bvl, 0.0):
        shared["bvl"] = bvl.astype(bf)
    gbi = bool(np.allclose(g, 1.0) and np.allclose(bb, 0.0))
    if not gbi:
        shared["gcol"] = g.reshape(HC, 128).T.astype(np.float32)
        shared["bcol"] = bb.reshape(HC, 128).T.astype(np.float32)

    starts_all = span_ids[..., 0].astype(np.float32)
    lens_all = (span_ids[..., 1] - span_ids[..., 0]).astype(np.float32)
    ends_all = starts_all + lens_all

    plan = _plan_cores(span_ids, masks)
    in_maps = []
    for b, pj in plan:
        n = len(pj)
        assert n <= spc
        if n:
            smin = int(starts_all[b, pj].min())
            tok0 = min(smin, T - tw)
        else:
            tok0 = 0
        sem = np.zeros((1, 3 * spc + 128), dtype=np.float32)
        sem[0, :n] = starts_all[b, pj] - tok0
        sem[0, spc:spc + n] = ends_all[b, pj] - tok0
        sem[0, 2 * spc:2 * spc + n] = 1.0
        sem[0, 3 * spc:] = 1.0
        m = dict(shared)
        m["xT"] = np.ascontiguousarray(xT_full[b][:, :, tok0:tok0 + tw])
        m["sem"] = sem
        in_maps.append(m)
    return in_maps, plan


def kernel(**inputs) -> np.ndarray:
    global _COMPILED
    from concourse.bass_utils import run_bass_kernel_spmd

    span_ids = np.asarray(inputs["span_ids"]).astype(np.int64)
    masks = np.asarray(inputs["span_masks"]).astype(bool)
    plan = _plan_cores(span_ids, masks)
    need = max((len(pj) for _, pj in plan), default=0)
    spc = max(416, ((need + 15) // 16) * 16)
    starts = span_ids[..., 0]
    ends = span_ids[..., 1]
    width = 1
    for b, pj in plan:
        if len(pj):
            width = max(width, int(ends[b, pj].max() - starts[b, pj].min()))
    tw = 256 if width <= 256 else (384 if width <= 384 else 512)

    key = (spc, tw)
    if key not in _COMPILED:
        gbi = (np.allclose(np.asarray(inputs["norm_g"], dtype=np.float32), 1.0)
               and np.allclose(np.asarray(inputs["norm_b"], dtype=np.float32), 0.0))
        in_b = np.asarray(inputs["in_proj_b"], dtype=np.float32)
        q0 = np.asarray(inputs["dummy_query"], dtype=np.float32)
        in_w = np.asarray(inputs["in_proj_w"], dtype=np.float32)
        bk = in_b[H:2 * H]
        qs = (q0 @ in_w[0:H].T + in_b[0:H]) / np.sqrt(HD)
        constv = np.array([qs[n * HD:(n + 1) * HD] @ bk[n * HD:(n + 1) * HD]
                           for n in range(NH)])
        ibz = bool(np.allclose(in_b[2 * H:], 0.0) and np.allclose(constv, 0.0))
        b2z = bool(np.allclose(np.asarray(inputs["ffn_b2"], dtype=np.float32), 0.0))
        _COMPILED[key] = _build(spc, tw, gb_identity=gbi, ib_zero=ibz,
                                b2_zero=b2z)
    nc = _COMPILED[key]

    in_maps, plan = _host_prepare(inputs, spc, tw)
    res = run_bass_kernel_spmd(nc, in_maps, core_ids=list(range(NCORES)))
    full = np.zeros((B, S, H), dtype=np.float32)
    for core in range(NCORES):
        b, pj = plan[core]
        if len(pj) == 0:
            continue
        arr = np.asarray(res.results[core]["out"]).astype(np.float32)
        # arr[p, c, s] -> out[s, c*128+p]
        o = arr.transpose(2, 1, 0).reshape(arr.shape[2], H)
        full[b, pj] = o[:len(pj)]
    return full
